# revision 1
# baseline (speedup 1.0000x reference)
"""Trainium2 Bass kernel for nn_MHSA_CGLU (PSA attention + Convolutional GLU).

Sharding: data-parallel over batch (B=8), one NeuronCore per batch element.
Activations in [channels, N=H*W] layout (channels on SBUF partitions).

v2 structure:
- all matmul operands bf16 (FWL weight loads), biases as rank-1 matmuls
- q/k packed 4 heads/tile at 32-aligned partitions -> row-group-concurrent
  s-matmuls (tile_position)
- exp(S) split between ScalarE (table exp) and DVE (Schraudolph bit-trick:
  round(x*c1+c2) as int16 == bf16 bits of exp(x); ~3.7% elementwise but
  cancels through softmax normalization to ~1e-3 final)
- softmax denominators via ones-column in v^T, reciprocal computed in a
  DMA-reshaped [128,64] layout
- 3x3 depthwise convs as 9 diagonal matmuls (host-precomputed bf16 diags)
- software-pipelined emission: s/exp of pair p overlaps o-matmuls of p-1
  and pe-dwconv fillers
"""

import ml_dtypes
import numpy as np

import concourse.bass as bass  # noqa: F401
import concourse.mybir as mybir
import concourse.tile as tile
from concourse import bacc
from concourse.bass_utils import run_bass_kernel_spmd

F32 = mybir.dt.float32
F32R = mybir.dt.float32r
BF16 = mybir.dt.bfloat16
I16 = mybir.dt.int16
U32 = mybir.dt.uint32
AF = mybir.ActivationFunctionType
OP = mybir.AluOpType

EPS = 1e-5
NH, KD, HD = 8, 16, 32
C, N, HH, WW = 256, 1024, 32, 32
HID = 170
SCALE = KD ** -0.5

# Schraudolph exp -> bf16 bits via int16: round(x*EC1 + EC2)
EC1 = float(np.log2(np.e) * 128.0)
EC2 = float(127.0 * 128.0 - 4.7)

# (pair, mt) steps where the ODD head's exp tile runs on DVE (Schraudolph)
# instead of ScalarE; the even head always uses ScalarE so it never idles.
EXP_DVE = ({(p, mt) for p in range(4) for mt in (1, 3, 5, 7)}
           | {(0, 2), (2, 2)})


# --------------------------------------------------------------------------
# Host-side parameter folding
# --------------------------------------------------------------------------

def _bn_fold(p):
    g, b, m, v = [np.asarray(a, np.float64) for a in p]
    s = g / np.sqrt(v + EPS)
    return s, b - s * m


def fold_consts(inp):
    f64 = lambda a: np.asarray(a, np.float64)
    ln1_g, ln1_b = f64(inp["ln1_g"]), f64(inp["ln1_b"])
    ln2_g, ln2_b = f64(inp["ln2_g"]), f64(inp["ln2_b"])

    # qkv conv + BN, with LN1 affine folded in.
    s_qkv, b_qkv = _bn_fold(inp["qkv_bn"])
    Wq = s_qkv[:, None] * f64(inp["qkv_w"])          # [512, 256]
    bq = b_qkv.copy()
    bq += Wq @ ln1_b
    Wq = Wq * ln1_g[None, :]

    q_rows = np.concatenate([np.arange(64 * h, 64 * h + 16) for h in range(NH)])
    k_rows = q_rows + 16
    v_rows = np.concatenate([np.arange(64 * h + 32, 64 * h + 64) for h in range(NH)])
    Wq_q, bq_q = Wq[q_rows] * SCALE, bq[q_rows] * SCALE
    Wq_k, bq_k = Wq[k_rows], bq[k_rows]
    Wq_v, bq_v = Wq[v_rows], bq[v_rows]

    # qkv M-tiles: Q0(h0-3), Q1(h4-7), K0, K1 (head j at cols 32j..32j+16,
    # rest zero), V0, V1 dense.
    Wfull = np.zeros((6, 128, 256))
    biasqk = np.zeros((1, 4, 128))
    for h in range(NH):
        T, j = divmod(h, 4)
        sl = slice(32 * j, 32 * j + 16)
        Wfull[T][sl] = Wq_q[16 * h: 16 * h + 16]
        biasqk[0, T, sl] = bq_q[16 * h: 16 * h + 16]
        Wfull[2 + T][sl] = Wq_k[16 * h: 16 * h + 16]
        biasqk[0, 2 + T, sl] = bq_k[16 * h: 16 * h + 16]
    Wfull[4] = Wq_v[0:128]
    Wfull[5] = Wq_v[128:256]
    # SBUF layout [part(cin%128), kt(cin//128), 6*128 m-cols]
    wqkvT = np.ascontiguousarray(
        Wfull.reshape(768, 256).T.reshape(2, 128, 768).transpose(1, 0, 2))
    bqv_row = bq_v.reshape(1, 256)

    # v^T conv: [n, 33h+d]; col 33h+32 is the ones column (zero weight;
    # ones added via rank-1 matmul with onescol264).
    WvT = np.zeros((256, 264))
    for h in range(NH):
        WvT[:, 33 * h: 33 * h + 32] = Wq_v[32 * h: 32 * h + 32].T
    wvT = np.ascontiguousarray(WvT.reshape(2, 128, 264).transpose(1, 0, 2))
    onescol264 = np.zeros((1, 264))
    onescol264[0, 32::33] = 1.0

    # pe branch dwconv taps (BN scale folded); o2 + bq_v + b_pe folded
    # through proj into its bias.
    s_pe, b_pe = _bn_fold(inp["pe_bn"])
    taps_pe = s_pe[:, None, None] * f64(inp["pe_w"])[:, 0]     # [256, 3, 3]
    bfold_pe = b_pe + bq_v

    s_pr, b_pr = _bn_fold(inp["proj_bn"])
    Wpr = s_pr[:, None] * f64(inp["proj_w"])
    bias_proj = (b_pr + Wpr @ bfold_pe).reshape(1, 256)
    wprojT = np.ascontiguousarray(Wpr.T.reshape(2, 128, 256).transpose(1, 0, 2))

    # fc1 with LN2 affine folded; M-tiles A1(128) A2(42) G1(128) G2(42)
    W1 = f64(inp["fc1_w"])
    b1 = f64(inp["fc1_b"]) + W1 @ ln2_b
    W1 = W1 * ln2_g[None, :]
    W1cols = np.zeros((256, 512))
    b1cols = np.zeros((1, 4, 128))
    W1cols[:, 0:128] = W1[0:128].T;        b1cols[0, 0, 0:128] = b1[0:128]
    W1cols[:, 128:170] = W1[128:170].T;    b1cols[0, 1, 0:42] = b1[128:170]
    W1cols[:, 256:384] = W1[170:298].T;    b1cols[0, 2, 0:128] = b1[170:298]
    W1cols[:, 384:426] = W1[298:340].T;    b1cols[0, 3, 0:42] = b1[298:340]
    wfc1T = np.ascontiguousarray(W1cols.reshape(2, 128, 512).transpose(1, 0, 2))

    taps_dw = f64(inp["dw_w"])[:, 0]                            # [170, 3, 3]
    b_dw = f64(inp["dw_b"])

    W2 = f64(inp["fc2_w"])                                      # [256, 170]
    W2T = np.zeros((2, 128, 256))
    W2T[0] = W2[:, 0:128].T
    W2T[1, 0:42] = W2[:, 128:170].T
    wfc2T = np.ascontiguousarray(W2T.transpose(1, 0, 2))        # [128, 2, 256]
    bfin_row = (f64(inp["fc2_b"]) + ln2_b).reshape(1, 256)

    # compact tap columns [128, (2 pe-tiles + 2 dw-tiles), 9 taps]; diag
    # matrices are built on-device by GPSIMD from id128 * tap column
    taps = np.zeros((128, 4, 9))
    for tap in range(9):
        dy, dx = divmod(tap, 3)
        taps[:, 0, tap] = taps_pe[0:128, dy, dx]
        taps[:, 1, tap] = taps_pe[128:256, dy, dx]
        taps[0:128, 2, tap] = taps_dw[0:128, dy, dx]
        taps[0:42, 3, tap] = taps_dw[128:170, dy, dx]

    # per-partition columns: 0 = b_dw (gelu bias), 1 = ln2_g (xn2 scale)
    pvec = np.zeros((128, 2, 2))
    pvec[0:128, 0, 0] = b_dw[0:128]
    pvec[0:42, 1, 0] = b_dw[128:170]
    pvec[:, 0, 1], pvec[:, 1, 1] = ln2_g[0:128], ln2_g[128:256]

    ind = np.zeros((8, 256))
    for h in range(NH):
        ind[h, 32 * h: 32 * h + 32] = 1.0

    # stat lhsT columns, replicated to M=33 so the psum stat rows 0..32 are
    # all written (rows 1..31 are dummies; row 0 = chunk0, row 32 = chunk1)
    statcol33 = np.zeros((128, 2, 33))
    statcol33[:, 0, :] = -1.0 / C
    statcol33[:, 1, :] = 1.0 / C

    # ---- pack every bf16 constant into one [128, BLOB_COLS] blob ----
    # [128, X] consts occupy all rows; [1/8, X] row-consts live in the top
    # rows of their column range. Offsets must match BLOB_SLOTS below.
    blob = np.zeros((128, BLOB_COLS))
    arrs = {
        "statcol33": statcol33.reshape(128, -1),
        "ones128": np.ones((128, 128)),
        "wqkvT": wqkvT.reshape(128, -1),
        "wvT": wvT.reshape(128, -1),
        "biasqk": biasqk.reshape(1, -1),
        "bqv_row": bqv_row,
        "onescol264": onescol264,
        "ones_row": np.ones((1, 512)),
        "id128": np.eye(128),
        "wprojT": wprojT.reshape(128, -1),
        "wfc1T": wfc1T.reshape(128, -1),
        "wfc2T": wfc2T.reshape(128, -1),
        "ind": ind,
        "bias_proj": bias_proj,
        "biasfc1": b1cols.reshape(1, -1),
        "bfin_row": bfin_row,
    }
    off = 0
    for nm, rows, sh in BLOB_SLOTS:
        a = arrs[nm]
        c = a.shape[1]
        assert c == int(np.prod(sh)), (nm, c, sh)
        blob[0:rows, off:off + c] = a
        off += c
    assert off == BLOB_COLS, off

    f32 = lambda a: np.ascontiguousarray(a, dtype=np.float32)
    bf16 = lambda a: np.ascontiguousarray(a, dtype=ml_dtypes.bfloat16)
    return {
        "blob": bf16(blob),
        "pvec": f32(pvec),
        "taps": f32(taps),
        "epscol": f32(np.full((128, 1), EPS)),
    }


# --------------------------------------------------------------------------
# Device program (one core, one batch)
# --------------------------------------------------------------------------

# (name, rows, free-shape) laid out contiguously in the bf16 blob.
# Early group (needed by LN1/qkv/vT/attention) first so it can arrive in a
# separate first DMA; tail weights arrive second.
BLOB_SLOTS = [
    ("statcol33", 128, [2, 33]), ("ones128", 128, [128]),
    ("wqkvT", 128, [2, 768]), ("wvT", 128, [2, 264]),
    ("biasqk", 1, [4, 128]), ("bqv_row", 1, [256]),
    ("onescol264", 1, [264]), ("ones_row", 1, [512]),
    # ---- EARLY_COLS boundary ----
    ("id128", 128, [128]),
    ("wprojT", 128, [2, 256]), ("wfc1T", 128, [2, 512]),
    ("wfc2T", 128, [2, 256]),
    ("ind", 8, [256]),
    ("bias_proj", 1, [256]), ("biasfc1", 1, [4, 128]),
    ("bfin_row", 1, [256]),
]
EARLY_COLS = 66 + 128 + 1536 + 528 + 512 + 256 + 264 + 512
BLOB_COLS = sum(int(np.prod(sh)) for _, _, sh in BLOB_SLOTS)


def _ln(nc, work, rows, psS, psO, x_tiles, xb, consts, z_tiles, emit_dummy=None, gp_half=False):
    """LayerNorm over channels. x_tiles: 2x[128,N] f32r; xb: bf16 copies
    (written here). Writes z_tiles (bf16): z = (x - mu) * rstd."""
    for t in range(2):
        nc.vector.tensor_copy(xb[t][:], x_tiles[t][:])
    xsq = [work.tile([128, N], BF16, tag=f"xsq{t}", name=f"xsq{t}") for t in range(2)]
    for t in range(2):
        nc.vector.tensor_tensor(xsq[t][:], xb[t][:], xb[t][:], OP.mult)

    # stats psum tile: bank0 = -mean rows, bank1 = E[x^2] rows: chunk c0 via
    # M=33 matmul (rows 0..32 all written = valid), chunk c1 overwrites row 32.
    sp = psO.tile([128, N], F32, tag="psO", name="ln_stats")
    mcol33 = consts["statcol33"][:, 0, :]
    ecol33 = consts["statcol33"][:, 1, :]
    for t in range(2):
        nc.tensor.matmul(sp[0:33, 0:512], mcol33[:], xb[t][:, 0:512],
                         start=(t == 0), stop=(t == 1))
    for t in range(2):
        nc.tensor.matmul(sp[32:33, 0:512], mcol33[:, 0:1], xb[t][:, 512:1024],
                         start=(t == 0), stop=(t == 1))
    for t in range(2):
        nc.tensor.matmul(sp[0:33, 512:1024], ecol33[:], xsq[t][:, 0:512],
                         start=(t == 0), stop=(t == 1))
    for t in range(2):
        nc.tensor.matmul(sp[32:33, 512:1024], ecol33[:, 0:1], xsq[t][:, 512:1024],
                         start=(t == 0), stop=(t == 1))

    if emit_dummy is not None:
        emit_dummy(8)
    # row math on [33, 512]: rows 0 (chunk0) and 32 (chunk1) are live.
    msb = rows.tile([33, 512], F32R, tag="msb", name="ln_msb")
    nc.vector.tensor_copy(msb[:], sp[0:33, 0:512])          # -mu
    mu2 = rows.tile([33, 512], F32R, tag="mu2", name="ln_mu2")
    nc.vector.tensor_tensor(mu2[:], msb[:], msb[:], OP.mult)
    var = rows.tile([33, 512], F32R, tag="var", name="ln_var")
    nc.vector.tensor_tensor(var[:], sp[0:33, 512:1024], mu2[:], OP.subtract)
    nc.scalar.activation(var[:], var[:], AF.Ln, bias=consts["epscol"][0:33])
    A = rows.tile([33, 512], BF16, tag="A", name="ln_A")
    nc.scalar.activation(A[:], var[:], AF.Exp, scale=-0.5)  # rstd
    Br = rows.tile([33, 512], BF16, tag="Br", name="ln_Br")
    nc.vector.tensor_tensor(Br[:], msb[:], A[:], OP.mult)   # -mu*rstd

    # broadcast per chunk: bc = [A_c | Br_c] in one psum tile
    ones = consts["ones128"]
    absb = []
    for c in range(2):
        r = 32 * c
        bc = psS.tile([128, N], F32, tag="psS", name=f"ln_bc{c}")
        nc.tensor.matmul(bc[:, 0:512], ones[r:r + 1, 0:128], A[r:r + 1, :],
                         start=True, stop=True)
        nc.tensor.matmul(bc[:, 512:1024], ones[r:r + 1, 0:128], Br[r:r + 1, :],
                         start=True, stop=True)
        Ac = work.tile([128, 512], BF16, tag=f"Ac{c}", name=f"ln_Ac{c}")
        Bc = work.tile([128, 512], BF16, tag=f"Bc{c}", name=f"ln_Bc{c}")
        nc.scalar.copy(Ac[:], bc[:, 0:512])
        nc.scalar.copy(Bc[:], bc[:, 512:1024])
        absb.append((Ac, Bc))
        if emit_dummy is not None:
            emit_dummy(3)

    for t in range(2):
        for c in range(2):
            sl = slice(512 * c, 512 * c + 512)
            Ac, Bc = absb[c]
            eng = nc.gpsimd if (gp_half and t == 1) else nc.vector
            eng.tensor_tensor(z_tiles[t][:, sl], xb[t][:, sl], Ac[:], OP.mult)
            eng.tensor_tensor(z_tiles[t][:, sl], z_tiles[t][:, sl], Bc[:], OP.add)


def build(num_devices=8, debug_outs=False):
    nc = bacc.Bacc("TRN2", target_bir_lowering=False, debug=False,
                   num_devices=num_devices)

    x_d = nc.dram_tensor("x", [C, N], F32R, kind="ExternalInput")
    blob_d = nc.dram_tensor("blob", [128, BLOB_COLS], BF16, kind="ExternalInput")
    pvec_d = nc.dram_tensor("pvec", [128, 2, 2], F32, kind="ExternalInput")
    taps_d = nc.dram_tensor("taps", [128, 4, 9], F32, kind="ExternalInput")
    epscol_d = nc.dram_tensor("epscol", [128, 1], F32, kind="ExternalInput")
    y_d = nc.dram_tensor("y", [C, N], F32, kind="ExternalOutput")
    dbg = {}
    if debug_outs:
        for nm, sh, dt in [("d_z1", [128, N], BF16), ("d_q0", [128, N], BF16),
                           ("d_k0", [128, N], BF16), ("d_pt00", [128, N], BF16),
                           ("d_oall0", [128, N], BF16), ("d_rrow", [8, N], F32),
                           ("d_o20", [128, N], BF16), ("d_xattn0", [128, N], F32)]:
            dbg[nm] = nc.dram_tensor(nm, sh, dt, kind="ExternalOutput")

    with tile.TileContext(nc) as tc:
        with tc.tile_pool(name="singles", bufs=1) as singles, \
             tc.tile_pool(name="work", bufs=1) as work, \
             tc.tile_pool(name="rows", bufs=2) as rows, \
             tc.tile_pool(name="ptp", bufs=34) as ptp, \
             tc.tile_pool(name="stg", bufs=2) as stg, \
             tc.tile_pool(name="psS", bufs=2, space="PSUM") as psS, \
             tc.tile_pool(name="psO", bufs=1, space="PSUM") as psO, \
             tc.tile_pool(name="psD", bufs=1, space="PSUM") as psD:

            # ---- input first, then constants (two blob DMAs on
            # different queues so the early group lands fast) ----
            xt = [work.tile([128, N], F32R, tag=f"x{t}", name=f"x{t}") for t in range(2)]
            nc.sync.dma_start(xt[0][:], x_d.ap()[0:128, :])
            nc.scalar.dma_start(xt[1][:], x_d.ap()[128:256, :])
            blob = singles.tile([128, BLOB_COLS], BF16, tag="blob", name="blob")
            nc.sync.dma_start(blob[:, 0:EARLY_COLS], blob_d.ap()[:, 0:EARLY_COLS])
            nc.scalar.dma_start(blob[:, EARLY_COLS:], blob_d.ap()[:, EARLY_COLS:])
            pvec_t = singles.tile([128, 2, 2], F32, tag="pvec", name="pvec")
            nc.scalar.dma_start(pvec_t[:], pvec_d.ap())
            taps_t = singles.tile([128, 4, 9], F32, tag="taps", name="taps")
            nc.scalar.dma_start(taps_t[:], taps_d.ap())
            epscol_t = singles.tile([128, 1], F32, tag="epscol", name="epscol")
            nc.scalar.dma_start(epscol_t[:], epscol_d.ap())

            consts = {"pvec": pvec_t, "epscol": epscol_t, "taps": taps_t}
            _off = 0
            for _nm, _rows, _sh in BLOB_SLOTS:
                _c = int(np.prod(_sh))
                _v = blob[0:_rows, _off:_off + _c]
                if len(_sh) == 2:
                    _v = _v.rearrange("p (a b) -> p a b", a=_sh[0])
                elif len(_sh) == 3:
                    _v = _v.rearrange("p (a b c) -> p a b c", a=_sh[0], b=_sh[1])
                consts[_nm] = _v
                _off += _c

            # warm the ln/exp activation table off the critical path
            actwarm = work.tile([1, 1], F32, tag="actwarm", name="actwarm")
            nc.scalar.activation(actwarm[:], epscol_t[0:1, :], AF.Ln)

            # keep-warm dummies: the PE HAM clock-gate drops to 1.2 GHz
            # whenever array duty dips; these K=128 N=512 matmuls into a
            # never-read psum bank fill PE-idle windows so real matmuls run
            # at 2.4 GHz. Results are never consumed.
            dumref = [psD.tile([128, 1024], F32, tag="psD", name="dum")]

            def emit_dummy(k):
                for _ in range(k):
                    nc.tensor.matmul(dumref[0][:, 0:512], blob[:, 0:128],
                                     blob[:, 0:512], start=True, stop=True,
                                     skip_group_check=True)

            xb = [work.tile([128, N], BF16, tag=f"xb{t}", name=f"xb{t}") for t in range(2)]

            ones_row = consts["ones_row"]

            # padded dwconv inputs [128, 34, 36]; interior rows 1:33, cols 2:34
            vpad = [work.tile([128, 34, 36], BF16, tag=f"vpad{t}", name=f"vpad{t}")
                    for t in range(2)]
            apad = [work.tile([128, 34, 36], BF16, tag=f"apad{t}", name=f"apad{t}")
                    for t in range(2)]
            for t in range(2):
                nc.gpsimd.memset(vpad[t][:].bitcast(U32), 0)
                nc.gpsimd.memset(apad[t][:].bitcast(U32), 0)
            recip_row = work.tile([8, N], BF16, tag="recip_row", name="recip_row")
            nc.gpsimd.memset(recip_row[:].bitcast(U32), 0)

            emit_dummy(10)

            # diag tap matrices built by (otherwise idle) GPSIMD:
            # diag[t][tap] = id128 * tapcol
            diags = []
            for t in range(4):
                n_rows = 42 if t == 3 else 128
                per = []
                for tap in range(9):
                    dg = work.tile([128, 128], BF16, tag=f"dg{t}_{tap}",
                                   name=f"dg{t}_{tap}")
                    nc.gpsimd.tensor_scalar(
                        dg[0:n_rows], consts["id128"][0:n_rows],
                        consts["taps"][0:n_rows, t, tap:tap + 1], None, OP.mult)
                    per.append(dg)
                diags.append(per)

            # ---- LN1 ----
            z1 = [work.tile([128, N], BF16, tag=f"z1_{t}", name=f"z1_{t}") for t in range(2)]
            _ln(nc, work, rows, psS, psO, xt, xb, consts, z1, emit_dummy)

            # ---- qkv conv: M-tiles Q0 Q1 K0 K1 V0 V1 ----
            qk_sb = []
            for mt in range(6):
                ps = psS.tile([128, N], F32, tag="psS", name=f"qkv{mt}")
                for c in range(2):
                    sl = slice(c * 512, (c + 1) * 512)
                    for kt in range(2):
                        nc.tensor.matmul(
                            ps[:, sl], consts["wqkvT"][:, kt, mt * 128:(mt + 1) * 128],
                            z1[kt][:, sl], start=(kt == 0), stop=False)
                    if mt < 4:
                        nc.tensor.matmul(
                            ps[:, sl], consts["biasqk"][:, mt, :],
                            ones_row[:, 0:512], start=False, stop=True)
                    else:
                        nc.tensor.matmul(
                            ps[:, sl], consts["bqv_row"][:, (mt - 4) * 128:(mt - 3) * 128],
                            ones_row[:, 0:512], start=False, stop=True)
                if mt < 4:
                    t_sb = work.tile([128, N], BF16, tag=f"qk{mt}", name=f"qk{mt}")
                    if mt < 2:
                        nc.scalar.copy(t_sb[:], ps[:])
                    else:
                        nc.vector.tensor_copy(t_sb[:], ps[:])
                    qk_sb.append(t_sb)
                else:
                    nc.scalar.copy(vpad[mt - 4][:, 1:33, 2:34], ps[:])
            q_sb, k_sb = qk_sb[0:2], qk_sb[2:4]

            # ---- v^T conv ----
            vT_sb = []
            for nt in range(8):
                ps = psS.tile([128, 264], F32, tag="psS", name=f"vT{nt}")
                for kt in range(2):
                    nc.tensor.matmul(
                        ps[:], z1[kt][:, nt * 128:(nt + 1) * 128],
                        consts["wvT"][:, kt, :], start=(kt == 0), stop=False)
                nc.tensor.matmul(ps[:], ones_row[0:1, 0:128],
                                 consts["onescol264"][:], start=False, stop=True)
                t_sb = work.tile([128, 264], BF16, tag=f"vT{nt}", name=f"vT{nt}")
                nc.vector.tensor_copy(t_sb[:], ps[:])
                vT_sb.append(t_sb)

            # ---- attention (pipelined over head pairs) ----
            # pair p: heads (2p, 2p+1); head h: tile h//4, row group 32*(h%4)
            pts = {}        # (h, mt) -> bf16 [128, N]
            o_all = [work.tile([128, N], BF16, tag=f"oall{t}", name=f"oall{t}")
                     for t in range(2)]
            o2e = [work.tile([128, N], BF16, tag=f"o2{t}", name=f"o2{t}")
                   for t in range(2)]
            r128 = work.tile([128, 64], BF16, tag="r128", name="r128")
            pe_sb = [work.tile([128, N], BF16, tag=f"pe{t}", name=f"pe{t}")
                     for t in range(2)]
            stage_tiles = {}
            o_ps = {}

            def emit_s_exp(p, mt):
                # mixed-head tiles: tile `c` holds [h0 n-chunk c | h1 n-chunk c]
                # in its two banks; the two matmuls use different row groups
                # and different banks, so they run concurrently in the array.
                h0, h1 = 2 * p, 2 * p + 1
                T = p // 2
                g0, g1 = 32 * (h0 % 4), 32 * (h1 % 4)
                msl = slice(mt * 128, (mt + 1) * 128)
                for c in range(2):
                    sl = slice(c * 512, (c + 1) * 512)
                    sm = psS.tile([128, N], F32, tag="psS", name=f"s{p}_{mt}_{c}")
                    nc.tensor.matmul(sm[:, 0:512], k_sb[T][g0:g0 + 16, msl],
                                     q_sb[T][g0:g0 + 16, sl],
                                     start=True, stop=True, tile_position=(g0, 0))
                    nc.tensor.matmul(sm[:, 512:1024], k_sb[T][g1:g1 + 16, msl],
                                     q_sb[T][g1:g1 + 16, sl],
                                     start=True, stop=True, tile_position=(g1, 0))
                    pt = ptp.tile([128, N], BF16, tag="pt", name=f"pt{p}_{mt}_{c}")
                    if c == 1:
                        nc.vector.tensor_scalar(
                            pt[:].bitcast(I16), sm[:], EC1, EC2, OP.mult, OP.add)
                    else:
                        nc.scalar.activation(pt[:], sm[:], AF.Exp)
                    pts[(p, mt, c)] = pt

            def emit_o(p, mt):
                h0, h1 = 2 * p, 2 * p + 1
                if mt == 0:
                    o_ps[p] = psO.tile([128, N], F32, tag="psO", name=f"o{p}")
                ops = o_ps[p]
                for c in range(2):
                    sl = slice(c * 512, (c + 1) * 512)
                    ptm = pts[(p, mt, c)]
                    nc.tensor.matmul(
                        ops[0:33, sl], vT_sb[mt][:, 33 * h0: 33 * h0 + 33],
                        ptm[:, 0:512], start=(mt == 0), stop=(mt == 7),
                        tile_position=(0, 0))
                    nc.tensor.matmul(
                        ops[64:97, sl], vT_sb[mt][:, 33 * h1: 33 * h1 + 33],
                        ptm[:, 512:1024], start=(mt == 0), stop=(mt == 7),
                        tile_position=(0, 64))

            def emit_stage(p):
                h0, h1 = 2 * p, 2 * p + 1
                stage = stg.tile([97, N], BF16, tag="stage", name=f"stage{p}")
                if p == 3:
                    nc.scalar.copy(stage[:], o_ps[p][0:97, :])
                else:
                    nc.vector.tensor_copy(stage[:], o_ps[p][0:97, :])
                for hh, base in ((h0, 0), (h1, 64)):
                    oT, oj = divmod(hh, 4)
                    nc.sync.dma_start(o_all[oT][32 * oj: 32 * oj + 32, :],
                                      stage[base: base + 32, :])
                    nc.sync.dma_start(r128[16 * hh:16 * hh + 16, :],
                                      stage[base + 32: base + 33, :])
                stage_tiles[p] = stage

            def emit_pe_dwconv(t, taps):
                if t not in pe_ps_map:
                    pe_ps_map[t] = psS.tile([128, N], F32, tag="psS", name=f"pe_ps{t}")
                ps = pe_ps_map[t]
                for tap in taps:
                    dy, dx = divmod(tap, 3)
                    for c in range(2):
                        rhs = vpad[t][:, dy + 16 * c: dy + 16 * c + 16, dx + 1: dx + 33]
                        nc.tensor.matmul(
                            ps[:, c * 512:(c + 1) * 512],
                            diags[t][tap][:], rhs,
                            start=(tap == 0), stop=(tap == 8))
                if taps[-1] == 8:
                    nc.vector.tensor_copy(pe_sb[t][:], ps[:])

            def emit_recip_quarter(q):
                lo = 32 * q
                with nc.allow_low_precision(reason="softmax recip"):
                    nc.vector.reciprocal(recip128[lo:lo + 32, :], r128[lo:lo + 32, :])
                nc.sync.dma_start(recip_row[2 * q:2 * q + 2, :],
                                  recip128[lo:lo + 32, :])

            recip128 = work.tile([128, 64], BF16, tag="recip128", name="recip128")
            pe_ps_map = {}

            for p in range(4):
                for mt in range(8):
                    emit_s_exp(p, mt)
                    if p >= 1:
                        emit_o(p - 1, mt)
                    emit_dummy(2)
                if p >= 1:
                    emit_stage(p - 1)
                    emit_dummy(2)
                if p == 1:
                    emit_pe_dwconv(0, list(range(9)))
                if p == 1:
                    emit_recip_quarter(0)   # heads 0,1 (stage 0 done)
                if p == 2:
                    emit_pe_dwconv(1, list(range(9)))
                    emit_recip_quarter(1)   # heads 2,3
                if p == 3:
                    emit_recip_quarter(2)   # heads 4,5 (stage 2 done)
                    # heads 0-3 normalization can complete during attention:
                    # recipB0 uses the (rotated) dummy psum bank.
                    rb0 = psD.tile([128, N], F32, tag="psD", name="recipB0")
                    dumref[0] = rb0
                    for c in range(2):
                        sl = slice(c * 512, (c + 1) * 512)
                        nc.tensor.matmul(rb0[:, sl], consts["ind"][:, 0:128],
                                         recip_row[:, sl], start=True, stop=True)
                    nc.vector.tensor_tensor(o2e[0][:], o_all[0][:], rb0[:], OP.mult)
                    nc.vector.tensor_tensor(o2e[0][:], o2e[0][:], pe_sb[0][:], OP.add)
                    dum2 = psD.tile([128, N], F32, tag="psD", name="dum2")
                    dumref[0] = dum2
            for mt in range(8):
                emit_o(3, mt)
            emit_stage(3)
            emit_recip_quarter(3)   # heads 6,7
            emit_dummy(13)

            if debug_outs:
                nc.sync.dma_start(dbg["d_z1"].ap(), z1[0][:])
                nc.sync.dma_start(dbg["d_q0"].ap(), q_sb[0][:])
                nc.sync.dma_start(dbg["d_k0"].ap(), k_sb[0][:])
                nc.sync.dma_start(dbg["d_pt00"].ap(), pts[(0, 0, 0)][:])
                nc.sync.dma_start(dbg["d_oall0"].ap(), o_all[0][:])
                pass

            # ---- normalize + pe add for heads 4-7 (0-3 done in-attention) ----
            o2 = o2e
            for t in (1,):
                rb = psS.tile([128, N], F32, tag="psS", name=f"recipB{t}")
                for c in range(2):
                    sl = slice(c * 512, (c + 1) * 512)
                    nc.tensor.matmul(rb[:, sl], consts["ind"][:, t * 128:(t + 1) * 128],
                                     recip_row[:, sl], start=True, stop=True)
                nc.vector.tensor_tensor(o2[t][:], o_all[t][:], rb[:], OP.mult)
                nc.vector.tensor_tensor(o2[t][:], o2[t][:], pe_sb[t][:], OP.add)
                emit_dummy(7)

            if debug_outs:
                nc.sync.dma_start(dbg["d_o20"].ap(), o2[0][:])

            # ---- proj conv + residual (in place on x tiles) ----
            x_attn = xt
            for mt in range(2):
                ps = psS.tile([128, N], F32, tag="psS", name=f"proj{mt}")
                for c in range(2):
                    sl = slice(c * 512, (c + 1) * 512)
                    for kt in range(2):
                        nc.tensor.matmul(
                            ps[:, sl], consts["wprojT"][:, kt, mt * 128:(mt + 1) * 128],
                            o2[kt][:, sl], start=(kt == 0), stop=False)
                    nc.tensor.matmul(
                        ps[:, sl], consts["bias_proj"][:, mt * 128:(mt + 1) * 128],
                        ones_row[:, 0:512], start=False, stop=True)
                nc.vector.tensor_tensor(x_attn[mt][:], xt[mt][:], ps[:], OP.add)
                emit_dummy(2)

            if debug_outs:
                nc.sync.dma_start(dbg["d_xattn0"].ap(), x_attn[0][:].bitcast(F32))

            # ---- LN2 ----
            z2 = [work.tile([128, N], BF16, tag=f"z2_{t}", name=f"z2_{t}") for t in range(2)]
            _ln(nc, work, rows, psS, psO, x_attn, xb, consts, z2, emit_dummy, gp_half=True)

            # ---- fc1: M-tiles A1(128) A2(42) G1(128) G2(42) ----
            g_ps = []
            nparts = [128, 42, 128, 42]
            fc1_pools = [(psS, "psS"), (psS, "psS"), (psO, "psO"), (psS, "psS")]
            for mt in range(4):
                npart = nparts[mt]
                pool, tagname = fc1_pools[mt]
                ps = pool.tile([128, N], F32, tag=tagname, name=f"fc1_{mt}")
                for c in range(2):
                    sl = slice(c * 512, (c + 1) * 512)
                    for kt in range(2):
                        nc.tensor.matmul(
                            ps[0:npart, sl],
                            consts["wfc1T"][:, kt, mt * 128: mt * 128 + npart],
                            z2[kt][:, sl], start=(kt == 0), stop=False)
                    nc.tensor.matmul(
                        ps[0:npart, sl],
                        consts["biasfc1"][:, mt, 0:npart],
                        ones_row[:, 0:512], start=False, stop=True)
                if mt < 2:
                    nc.scalar.copy(apad[mt][0:npart, 1:33, 2:34], ps[0:npart])
                else:
                    g_ps.append(ps)
                emit_dummy(2)

            # ---- GLU dwconv + gelu + gate ----
            da_ps = [psS.tile([128, N], F32, tag="psS", name=f"da{t}")
                     for t in range(2)]
            for tap in range(9):
                dy, dx = divmod(tap, 3)
                for t in range(2):
                    npart = nparts[t]
                    for c in range(2):
                        rhs = apad[t][0:npart, dy + 16 * c: dy + 16 * c + 16,
                                      dx + 1: dx + 33]
                        nc.tensor.matmul(
                            da_ps[t][0:npart, c * 512:(c + 1) * 512],
                            diags[2 + t][tap][0:npart, 0:npart], rhs,
                            start=(tap == 0), stop=(tap == 8))
            ag = []
            for t in range(2):
                npart = nparts[t]
                a_act = work.tile([128, N], BF16, tag=f"aact{t}", name=f"aact{t}")
                nc.scalar.activation(a_act[0:npart], da_ps[t][0:npart], AF.Gelu,
                                     bias=consts["pvec"][0:npart, t, 0:1])
                emit_dummy(2)
                agt = work.tile([128, N], BF16, tag=f"ag{t}", name=f"ag{t}")
                nc.vector.tensor_tensor(agt[0:npart], a_act[0:npart],
                                        g_ps[t][0:npart], OP.mult)
                ag.append(agt)
                emit_dummy(3)

            # ---- fc2 + final residuals ----
            for mt in range(2):
                ps = psS.tile([128, N], F32, tag="psS", name=f"fc2_{mt}")
                for c in range(2):
                    sl = slice(c * 512, (c + 1) * 512)
                    for kt in range(2):
                        npart = nparts[kt]
                        nc.tensor.matmul(
                            ps[:, sl],
                            consts["wfc2T"][0:npart, kt, mt * 128:(mt + 1) * 128],
                            ag[kt][0:npart, sl], start=(kt == 0), stop=False)
                    nc.tensor.matmul(
                        ps[:, sl], consts["bfin_row"][:, mt * 128:(mt + 1) * 128],
                        ones_row[:, 0:512], start=False, stop=False)
                # x_attn folded into the psum via identity matmuls (PE is
                # idle in the tail); u = g2*z2 + bfin on ScalarE; y = u + psum
                for c in range(2):
                    sl = slice(c * 512, (c + 1) * 512)
                    nc.tensor.matmul(ps[:, sl], consts["id128"][:],
                                     xb[mt][:, sl], start=False, stop=True,
                                     skip_group_check=True)
                ut = work.tile([128, N], BF16, tag=f"u{mt}", name=f"u{mt}")
                nc.scalar.mul(ut[:], z2[mt][:], consts["pvec"][:, mt, 1:2])
                yt = work.tile([128, N], F32, tag=f"y{mt}", name=f"y{mt}")
                nc.vector.tensor_tensor(yt[:], ut[:], ps[:], OP.add)
                nc.sync.dma_start(y_d.ap()[mt * 128:(mt + 1) * 128, :], yt[:])
                emit_dummy(2)

    nc.compile()
    return nc


_NC = None


def kernel(**inputs):
    global _NC
    consts = fold_consts(inputs)
    if _NC is None:
        _NC = build()
    x = np.asarray(inputs["x"], np.float32)
    B = x.shape[0]
    in_maps = []
    for b in range(B):
        m = dict(consts)
        m["x"] = np.ascontiguousarray(x[b].reshape(C, N))
        in_maps.append(m)
    res = run_bass_kernel_spmd(_NC, in_maps, core_ids=list(range(B)))
    out = np.stack([res.results[b]["y"].reshape(C, HH, WW) for b in range(B)])
    return out

